# revision 19
# baseline (speedup 1.0000x reference)
"""Trainium2 Bass kernel for the retrieval-kNN problem (B=512, C=1000, D=512, K=10).

Math (equivalent to the reference; the softmax is used unnormalized — the
rowmax shift inside the reference softmax cancels exactly, and the 1/S
normalization is applied after the distance matmul):
  E   = exp(-sqrt((sqrtN*ex2)^2 - ex1^2))          [C, D] softmax numerator
  S_c = sum_d E[c, d]                               softmax denominator
  usim[b,c] = sum_d x2[b,d] E[c,d] + x[b,d] W2[c,d] + 0.25 * cst4[c]
      with W2 = pm2 E  (pm2 = -2p fed from host),
           cst4 = sum_d pm2^2 E = 4 sum_d p^2 E
  simi = usim / S_c
  top-10 smallest simi per row -> conf = sum(vals)/vals,
  predict = proto_label[argmin simi]

Sharding: pure data parallel over the batch (64 rows per core); the per-class
matrices are replicated. Everything is fed host-transposed ([D, C]) so the
contraction dim d sits on partitions: no on-chip transposes, and the softmax
denominator falls out of the main matmul via an extra all-ones lhsT column.
No collectives (measured ~60us fixed overhead per NEFF for any NRT
collective on this platform).
"""
import sys

for _p in ("/opt/trn_rl_repo",):
    if _p not in sys.path:
        sys.path.insert(0, _p)

import numpy as np

B, C, D, K = 512, 1000, 512, 10
NCORES = 8
BL = B // NCORES   # 64 batch rows per core
DT = D // 128      # 4 d-tiles
SP = 512           # matmul column split (PSUM bank = 512 fp32)
CW = (SP, C - SP)  # 512, 488
BIG = float(1 << 24)

_CACHE = {}


def _make_tc(nc):
    import concourse.mybir as mybir
    from concourse import tile
    from concourse.vector_clock import ScopedClock

    class SplitDrainTileContext(tile.TileContext):
        # The stock tail Drain carries every outstanding sem wait; this
        # walrus build rejects instructions with more than one sync wait.
        # Keep one wait on the drain, put the rest on SP nops.
        def _drain_and_barrier(self, tick_clock, wait_clock):
            nc = self.nc
            drain_inst = nc.sync.drain()
            wait_clock.add_sem_waits(
                drain_inst.ins, ScopedClock({None: tick_clock.global_clock})
            )
            si = drain_inst.ins.sync_info
            waits = list(si.on_wait) if si and si.on_wait else []
            if len(waits) > 1:
                assert self.sems is not None
                name_to_sem = {s.name: s for s in self.sems.allocated().values()}
                drain_inst.ins.sync_info = mybir.SyncInfo(
                    on_wait=[waits[0]],
                    on_update=list(si.on_update) if si.on_update else [],
                )
                for w in waits[1:]:
                    nc.sync.nop()._wait_ge(name_to_sem[w.ant_name], w.wait_value)
            nc.all_engine_barrier()
            popped = nc._tile_sem_poison_stack.pop()
            assert popped is self._sem_poison
            nc.clear_and_free_semaphores(list(self.sems.allocated().values()))
            nc.all_engine_barrier()

    return SplitDrainTileContext(nc)


def _split_sync_waits(nc, limit=1):
    """Move excess sem waits onto NoOps inserted just before the owning
    instruction on the same engine (walrus rejects multi-wait instructions)."""
    import concourse.mybir as mybir

    ctr = [0]
    for fn in nc.m.functions:
        for bb in fn.blocks:
            insts = bb.instructions
            i = 0
            while i < len(insts):
                inst = insts[i]
                si = inst.sync_info
                waits = list(si.on_wait) if si and si.on_wait else []
                if len(waits) > limit:
                    keep = waits[-limit:]
                    excess = waits[:-limit]
                    inst.sync_info = mybir.SyncInfo(
                        on_wait=keep,
                        on_update=list(si.on_update) if si.on_update else [],
                    )
                    nops = []
                    for j in range(0, len(excess), limit):
                        nop = mybir.InstNoOp(
                            name=f"I-splitw-{ctr[0]}", ins=[], outs=[])
                        ctr[0] += 1
                        nop.engine = inst.engine
                        nop.sync_info = mybir.SyncInfo(
                            on_wait=excess[j:j + limit], on_update=[])
                        nops.append(nop)
                    insts[i:i] = nops
                    i += len(nops)
                i += 1


def build_program(split_waits=True):
    """Build the Bass program (same SPMD program for all 8 cores)."""
    import concourse.bass as bass
    import concourse.mybir as mybir

    dt = mybir.dt
    F32 = dt.float32
    Alu = mybir.AluOpType
    Act = mybir.ActivationFunctionType
    X = mybir.AxisListType.X

    nc = bass.Bass("TRN2", target_bir_lowering=False, debug=False,
                   num_devices=NCORES)

    # Replicated transposed per-class matrices; per-core xT slab (64 cols).
    # ne2T = sqrt(cls_num)*ex2 transposed; pm2T = -2*protos transposed.
    pm2T_d = nc.dram_tensor("pm2T", [D, C], F32, kind="ExternalInput").ap()
    ne2T_d = nc.dram_tensor("ne2T", [D, C], F32, kind="ExternalInput").ap()
    e1T_d = nc.dram_tensor("e1T", [D, C], F32, kind="ExternalInput").ap()
    xT_d = nc.dram_tensor("xT", [D, BL], F32, kind="ExternalInput").ap()
    label_d = nc.dram_tensor("labelr", [1, C], F32, kind="ExternalInput").ap()

    conf_d = nc.dram_tensor("conf", [BL, K], F32, kind="ExternalOutput").ap()
    pred_d = nc.dram_tensor("pred", [BL, 1], dt.int32, kind="ExternalOutput").ap()

    tc = _make_tc(nc)
    with tc:
        with tc.tile_pool(name="sbuf", bufs=1) as pool, \
             tc.tile_pool(name="psum", bufs=1, space="PSUM") as psum, \
             tc.tile_pool(name="dram", bufs=1, space="DRAM") as dram:

            # ---- loads, spread across engine DMA queues ----
            # tiles reused in place: ne2 -> t1 -> u -> E ; e1 -> s1 -> rd0 ->
            # W2 ; pm2 -> PPE4
            ne2, e1, pm2, xT = [], [], [], []
            for t in range(DT):
                rows = slice(t * 128, (t + 1) * 128)
                for lst, src, nm, q in ((ne2, ne2T_d, "ne2", nc.sync),
                                        (e1, e1T_d, "e1", nc.scalar),
                                        (pm2, pm2T_d, "pm2", nc.gpsimd)):
                    tl = pool.tile([128, C], F32, tag=f"{nm}_{t}")
                    q.dma_start(tl[:], src[rows, :])
                    lst.append(tl)
                xt = pool.tile([128, BL], F32, tag=f"xT{t}")
                nc.sync.dma_start(xt[:], xT_d[rows, :])
                xT.append(xt)
            labelB = pool.tile([BL, C], F32, tag="labelB")
            nc.scalar.dma_start(labelB[:], label_d.to_broadcast([BL, C]))

            # x2/ones lhsT: cols 0-63 x^2, col 64 ones -> S row;
            # x lhsT: cols 0-63 x, col 64 zero.
            x2a, xa = [], []
            for t in range(DT):
                x2 = pool.tile([128, BL + 1], F32, tag=f"x2a{t}")
                nc.vector.tensor_mul(x2[:, :BL], xT[t][:], xT[t][:])
                nc.vector.memset(x2[:, BL:], 1.0)
                x2a.append(x2)
                xz = pool.tile([128, BL + 1], F32, tag=f"xa{t}")
                nc.scalar.copy(xz[:, :BL], xT[t][:])
                nc.vector.memset(xz[:, BL:], 0.0)
                xa.append(xz)
            # rank-1 const row: 0.25 over batch cols (cst4 = 4*cst), 0 at S
            q_row = pool.tile([1, BL + 1], F32, tag="q_row")
            nc.vector.memset(q_row[:, :BL], 0.25)
            nc.vector.memset(q_row[:, BL:], 0.0)
            ones_col = pool.tile([128, 1], F32, tag="ones_col")
            nc.vector.memset(ones_col[:], 1.0)

            # ---- per-class chain, full-width [128, 1000] ops ----
            # gp: t1, s1, PPE4, ppe adds ; DVE: u, W2 ; ACT: sqrt, exp
            for t in range(DT):
                nc.gpsimd.tensor_mul(ne2[t][:], ne2[t][:], ne2[t][:])
                nc.gpsimd.tensor_mul(e1[t][:], e1[t][:], e1[t][:])
                nc.vector.tensor_sub(ne2[t][:], ne2[t][:], e1[t][:])
            for t in range(DT):
                nc.scalar.activation(e1[t][:], ne2[t][:], Act.Sqrt)
            E, W2 = ne2, e1
            for t in range(DT):
                nc.scalar.activation(E[t][:], e1[t][:], Act.Exp, scale=-1.0)
            for t in range(DT):
                nc.vector.tensor_mul(W2[t][:], pm2[t][:], E[t][:])
                nc.gpsimd.tensor_mul(pm2[t][:], pm2[t][:], W2[t][:])
            nc.gpsimd.tensor_add(pm2[0][:], pm2[0][:], pm2[1][:])
            nc.gpsimd.tensor_add(pm2[2][:], pm2[2][:], pm2[3][:])
            nc.gpsimd.tensor_add(pm2[0][:], pm2[0][:], pm2[2][:])

            # ---- cst4 row via thin ones-matmul; usim[65, 1024] ----
            cst_ps = psum.tile([1, 1024], F32, tag="cst_ps")
            cst = pool.tile([1, C], F32, tag="cst")
            usim = psum.tile([BL + 1, 1024], F32, tag="usim")
            for h in range(2):
                pcols = slice(h * SP, h * SP + CW[h])
                nc.tensor.matmul(cst_ps[:, pcols], lhsT=ones_col[:],
                                 rhs=pm2[0][:, pcols], start=True, stop=True)
                nc.vector.tensor_copy(cst[:, pcols], cst_ps[:, pcols])
                for t in range(DT):
                    nc.tensor.matmul(usim[:, pcols], lhsT=x2a[t][:],
                                     rhs=E[t][:, pcols],
                                     start=(t == 0), stop=False)
                for t in range(DT):
                    nc.tensor.matmul(usim[:, pcols], lhsT=xa[t][:],
                                     rhs=W2[t][:, pcols],
                                     start=False, stop=False)
                nc.tensor.matmul(usim[:, pcols], lhsT=q_row[:],
                                 rhs=cst[:, pcols], start=False, stop=True)

            # ---- -1/S via [125, 8]-shaped reciprocal (parallel lanes) ----
            s_dr = dram.tile([1, C], F32, tag="s_dr")
            nc.scalar.copy(cst[:, :C], usim[BL:BL + 1, :C])  # reuse cst tile? no
            # (copy S row to SBUF then bounce through DRAM to reshape)
            nc.sync.dma_start(s_dr[:], cst[:, :C])
            s_sq = pool.tile([125, 8], F32, tag="s_sq")
            nc.sync.dma_start(s_sq[:], s_dr[:].rearrange("1 (p f) -> p f", f=8))
            nc.vector.reciprocal(s_sq[:], s_sq[:])
            nc.vector.tensor_scalar_mul(s_sq[:], s_sq[:], -1.0)
            nnr_dr = dram.tile([1, C], F32, tag="nnr_dr")
            nc.sync.dma_start(nnr_dr[:].rearrange("1 (p f) -> p f", f=8),
                              s_sq[:])
            nrB = pool.tile([BL, C], F32, tag="nrB_sb")
            nc.sync.dma_start(nrB[:], nnr_dr[:].to_broadcast([BL, C]))
            nsimi = pool.tile([BL, C], F32, tag="nsimi")
            nc.vector.tensor_mul(nsimi[:], usim[:BL, :C], nrB[:])

            # ---- top-10 smallest distances (largest neg_simi) ----
            v1 = pool.tile([BL, 8], F32, tag="v1")
            nc.vector.max(v1[:], nsimi[:])
            ns2 = pool.tile([BL, C], F32, tag="ns2")
            nc.vector.match_replace(ns2[:], v1[:], nsimi[:], -3.0e38)
            v2 = pool.tile([BL, 8], F32, tag="v2")
            nc.vector.max(v2[:], ns2[:])

            # conf = S10/v computed on negated values: (-S10)/(-v) = S10/v
            tk = pool.tile([BL, K], F32, tag="tk")
            nc.vector.tensor_copy(tk[:, :8], v1[:])
            nc.vector.tensor_copy(tk[:, 8:K], v2[:, :K - 8])
            s10 = pool.tile([BL, 1], F32, tag="s10")
            nc.vector.tensor_reduce(s10[:], tk[:], axis=X, op=Alu.add)
            rv = pool.tile([BL, K], F32, tag="rv")
            nc.vector.reciprocal(rv[:], tk[:])
            conf = pool.tile([BL, K], F32, tag="conf")
            nc.vector.tensor_scalar_mul(conf[:], rv[:], s10[:])
            nc.sync.dma_start(conf_d[:], conf[:])

            # ---- predict: label of the (first) argmin distance ----
            # eqm = -BIG*(nsimi==min) ; masked = eqm + label ; rowmin + BIG
            eqm = pool.tile([BL, C], F32, tag="eqm")
            nc.vector.tensor_scalar(eqm[:], nsimi[:], v1[:, 0:1], -BIG,
                                    op0=Alu.is_equal, op1=Alu.mult)
            masked = pool.tile([BL, C], F32, tag="masked")
            nc.gpsimd.tensor_add(masked[:], eqm[:], labelB[:])
            predf = pool.tile([BL, 1], F32, tag="predf")
            nc.vector.tensor_reduce(predf[:], masked[:], axis=X, op=Alu.min)
            predb = pool.tile([BL, 1], F32, tag="predb")
            nc.vector.tensor_scalar_add(predb[:], predf[:], BIG)
            predi = pool.tile([BL, 1], dt.int32, tag="predi")
            nc.vector.tensor_copy(predi[:], predb[:])
            nc.sync.dma_start(pred_d[:], predi[:])

    if split_waits:
        _split_sync_waits(nc)
    return nc


def make_in_maps(x, protos, ex2, ex1, cls_num, proto_label):
    x = np.asarray(x, dtype=np.float32)
    protos = np.asarray(protos, dtype=np.float32)
    ex2 = np.asarray(ex2, dtype=np.float32)
    ex1 = np.asarray(ex1, dtype=np.float32)
    sqn = np.sqrt(np.asarray(cls_num).astype(np.float32))[:, None]
    label_f = np.ascontiguousarray(np.asarray(proto_label).astype(np.float32)[None, :])
    pm2T = np.ascontiguousarray((-2.0 * protos).T)
    ne2T = np.ascontiguousarray((sqn * ex2).T)
    e1T = np.ascontiguousarray(ex1.T)
    xT = np.ascontiguousarray(x.T)
    in_maps = []
    for g in range(NCORES):
        in_maps.append({
            "pm2T": pm2T,
            "ne2T": ne2T,
            "e1T": e1T,
            "xT": np.ascontiguousarray(xT[:, g * BL:(g + 1) * BL]),
            "labelr": label_f,
        })
    return in_maps


def assemble(results):
    conf = np.concatenate([results[g]["conf"] for g in range(NCORES)], axis=0)
    pred = np.concatenate([results[g]["pred"][:, 0] for g in range(NCORES)],
                          axis=0).astype(np.int32)
    return pred, conf


def kernel(x, protos, ex2, ex1, cls_num, proto_label, k_nearest=K,
           _trace=False, _tmpdir=None):
    from concourse.bass_utils import run_bass_kernel_spmd

    if "nc" not in _CACHE:
        _CACHE["nc"] = build_program()
    nc = _CACHE["nc"]
    in_maps = make_in_maps(x, protos, ex2, ex1, cls_num, proto_label)
    res = run_bass_kernel_spmd(nc, in_maps, core_ids=list(range(NCORES)),
                               trace=_trace, tmpdir=_tmpdir)
    _CACHE["last_res"] = res
    return assemble(res.results)


# revision 21
# speedup vs baseline: 1.1232x; 1.1232x over previous
"""Trainium2 Bass kernel for the retrieval-kNN problem (B=512, C=1000, D=512, K=10).

Math (equivalent to the reference; the softmax is used unnormalized — the
rowmax shift inside the reference softmax cancels exactly, and the 1/S
normalization is applied after the distance matmul):
  E   = exp(-sqrt((sqrtN*ex2)^2 - ex1^2))          [C, D] softmax numerator
  S_c = sum_d E[c, d]                               softmax denominator
  usim[b,c] = sum_d x2[b,d] E[c,d] + x[b,d] W2[c,d] + 0.25 * cst4[c]
      with W2 = pm2 E  (pm2 = -2p fed from host),
           cst4 = sum_d pm2^2 E = 4 sum_d p^2 E
  simi = usim / S_c
  top-10 smallest simi per row -> conf = sum(vals)/vals,
  predict = proto_label[argmin simi]

Sharding: pure data parallel over the batch (64 rows per core); the per-class
matrices are replicated. Everything is fed host-transposed ([D, C]) so the
contraction dim d sits on partitions: no on-chip transposes, and the softmax
denominator falls out of the main matmul via an extra all-ones lhsT column.
No collectives (measured ~60us fixed overhead per NEFF for any NRT
collective on this platform).
"""
import sys

for _p in ("/opt/trn_rl_repo",):
    if _p not in sys.path:
        sys.path.insert(0, _p)

import numpy as np

B, C, D, K = 512, 1000, 512, 10
NCORES = 8
BL = B // NCORES   # 64 batch rows per core
DT = D // 128      # 4 d-tiles
SP = 512           # matmul column split (PSUM bank = 512 fp32)
CW = (SP, C - SP)  # 512, 488
BIG = float(1 << 24)

_CACHE = {}


def _make_tc(nc):
    import concourse.mybir as mybir
    from concourse import tile
    from concourse.vector_clock import ScopedClock

    class SplitDrainTileContext(tile.TileContext):
        # The stock tail Drain carries every outstanding sem wait; this
        # walrus build rejects instructions with more than one sync wait.
        # Keep one wait on the drain, put the rest on SP nops.
        def _drain_and_barrier(self, tick_clock, wait_clock):
            nc = self.nc
            drain_inst = nc.sync.drain()
            wait_clock.add_sem_waits(
                drain_inst.ins, ScopedClock({None: tick_clock.global_clock})
            )
            si = drain_inst.ins.sync_info
            waits = list(si.on_wait) if si and si.on_wait else []
            if len(waits) > 1:
                assert self.sems is not None
                name_to_sem = {s.name: s for s in self.sems.allocated().values()}
                drain_inst.ins.sync_info = mybir.SyncInfo(
                    on_wait=[waits[0]],
                    on_update=list(si.on_update) if si.on_update else [],
                )
                for w in waits[1:]:
                    nc.sync.nop()._wait_ge(name_to_sem[w.ant_name], w.wait_value)
            nc.all_engine_barrier()
            popped = nc._tile_sem_poison_stack.pop()
            assert popped is self._sem_poison
            nc.clear_and_free_semaphores(list(self.sems.allocated().values()))
            nc.all_engine_barrier()

    return SplitDrainTileContext(nc)


def _split_sync_waits(nc, limit=1):
    """Move excess sem waits onto NoOps inserted just before the owning
    instruction on the same engine (walrus rejects multi-wait instructions)."""
    import concourse.mybir as mybir

    ctr = [0]
    for fn in nc.m.functions:
        for bb in fn.blocks:
            insts = bb.instructions
            i = 0
            while i < len(insts):
                inst = insts[i]
                si = inst.sync_info
                waits = list(si.on_wait) if si and si.on_wait else []
                if len(waits) > limit:
                    keep = waits[-limit:]
                    excess = waits[:-limit]
                    inst.sync_info = mybir.SyncInfo(
                        on_wait=keep,
                        on_update=list(si.on_update) if si.on_update else [],
                    )
                    nops = []
                    for j in range(0, len(excess), limit):
                        nop = mybir.InstNoOp(
                            name=f"I-splitw-{ctr[0]}", ins=[], outs=[])
                        ctr[0] += 1
                        nop.engine = inst.engine
                        nop.sync_info = mybir.SyncInfo(
                            on_wait=excess[j:j + limit], on_update=[])
                        nops.append(nop)
                    insts[i:i] = nops
                    i += len(nops)
                i += 1


def build_program(split_waits=True):
    """Build the Bass program (same SPMD program for all 8 cores)."""
    import concourse.bass as bass
    import concourse.mybir as mybir

    dt = mybir.dt
    F32 = dt.float32
    Alu = mybir.AluOpType
    Act = mybir.ActivationFunctionType
    X = mybir.AxisListType.X

    nc = bass.Bass("TRN2", target_bir_lowering=False, debug=False,
                   num_devices=NCORES)

    # Replicated transposed per-class matrices; per-core xT slab (64 cols).
    # ne2T = sqrt(cls_num)*ex2 transposed; pm2T = -2*protos transposed.
    pm2T_d = nc.dram_tensor("pm2T", [D, C], F32, kind="ExternalInput").ap()
    ne2T_d = nc.dram_tensor("ne2T", [D, C], F32, kind="ExternalInput").ap()
    e1T_d = nc.dram_tensor("e1T", [D, C], F32, kind="ExternalInput").ap()
    xT_d = nc.dram_tensor("xT", [D, BL], F32, kind="ExternalInput").ap()
    label_d = nc.dram_tensor("labelr", [1, C], F32, kind="ExternalInput").ap()

    conf_d = nc.dram_tensor("conf", [BL, K], F32, kind="ExternalOutput").ap()
    pred_d = nc.dram_tensor("pred", [BL, 1], dt.int32, kind="ExternalOutput").ap()

    tc = _make_tc(nc)
    with tc:
        with tc.tile_pool(name="sbuf", bufs=1) as pool, \
             tc.tile_pool(name="psum", bufs=1, space="PSUM") as psum, \
             tc.tile_pool(name="dram", bufs=1, space="DRAM") as dram:

            # ---- loads, spread across engine DMA queues ----
            # tiles reused in place: ne2 -> t1 -> u -> E ; e1 -> s1 -> rd0 ->
            # W2 ; pm2 -> PPE4
            qs = [nc.sync, nc.scalar, nc.gpsimd]
            ne2, e1, pm2, xT = [], [], [], []
            qi = 0
            for t in range(DT):
                rows = slice(t * 128, (t + 1) * 128)
                for lst, src, nm in ((ne2, ne2T_d, "ne2"), (e1, e1T_d, "e1"),
                                     (pm2, pm2T_d, "pm2")):
                    tl = pool.tile([128, C], F32, tag=f"{nm}_{t}")
                    qs[qi % 3].dma_start(tl[:], src[rows, :])
                    qi += 1
                    lst.append(tl)
                xt = pool.tile([128, BL], F32, tag=f"xT{t}")
                qs[qi % 3].dma_start(xt[:], xT_d[rows, :])
                qi += 1
                xT.append(xt)
            labelB = pool.tile([BL, C], F32, tag="labelB")
            nc.gpsimd.dma_start(labelB[:], label_d.to_broadcast([BL, C]))

            # x2/ones lhsT: cols 0-63 x^2, col 64 ones -> S row;
            # x lhsT: cols 0-63 x, col 64 zero.
            x2a, xa = [], []
            for t in range(DT):
                x2 = pool.tile([128, BL + 1], F32, tag=f"x2a{t}")
                nc.vector.tensor_mul(x2[:, :BL], xT[t][:], xT[t][:])
                nc.vector.memset(x2[:, BL:], 1.0)
                x2a.append(x2)
                xz = pool.tile([128, BL + 1], F32, tag=f"xa{t}")
                nc.vector.tensor_copy(xz[:, :BL], xT[t][:])
                nc.vector.memset(xz[:, BL:], 0.0)
                xa.append(xz)
            # rank-1 const row: 0.25 over batch cols (cst4 = 4*cst), 0 at S
            q_row = pool.tile([1, BL + 1], F32, tag="q_row")
            nc.vector.memset(q_row[:, :BL], 0.25)
            nc.vector.memset(q_row[:, BL:], 0.0)
            ones_col = pool.tile([128, 1], F32, tag="ones_col")
            nc.vector.memset(ones_col[:], 1.0)

            # ---- per-class chain, full-width [128, 1000] ops ----
            # gp: t1, PPE4 (partial), ppe adds ; DVE: u, W2 (partial);
            # ACT: e1^2 (Square), sqrt, exp — batched per function so the
            # activation table loads exactly three times (forced ordering).
            from concourse.tile import add_dep_helper
            sq_i, sqrt_i = [], []
            for t in range(DT):
                nc.gpsimd.tensor_mul(ne2[t][:], ne2[t][:], ne2[t][:])
                sq_i.append(
                    nc.scalar.activation(e1[t][:], e1[t][:], Act.Square))
            for t in range(DT):
                nc.vector.tensor_sub(ne2[t][:], ne2[t][:], e1[t][:])
            for t in range(DT):
                i = nc.scalar.activation(e1[t][:], ne2[t][:], Act.Sqrt)
                add_dep_helper(i.ins, sq_i[-1].ins, sync=False,
                               reason="batch Square before Sqrt")
                sqrt_i.append(i)
            E, W2 = ne2, e1
            exp_i = []
            for t in range(DT):
                i = nc.scalar.activation(E[t][:], e1[t][:], Act.Exp,
                                         scale=-1.0)
                add_dep_helper(i.ins, sqrt_i[-1].ins, sync=False,
                               reason="batch Sqrt before Exp")
                exp_i.append(i)
            # early S = sum_d E via thin accumulating matmuls (hides the
            # later 1/S reshaping dance under the main matmul phase)
            S_ps = psum.tile([1, 1024], F32, tag="S_ps")
            for t in range(DT):
                for h in range(2):
                    pcols = slice(h * SP, h * SP + CW[h])
                    nc.tensor.matmul(S_ps[:, pcols], lhsT=ones_col[:],
                                     rhs=E[t][:, pcols],
                                     start=(t == 0), stop=(t == DT - 1))
            for t in range(DT):
                w2_eng = nc.vector if t < 2 else nc.gpsimd
                ppe_eng = nc.gpsimd if t < 2 else nc.vector
                w2_eng.tensor_mul(W2[t][:], pm2[t][:], E[t][:])
                ppe_eng.tensor_mul(pm2[t][:], pm2[t][:], W2[t][:])
            nc.gpsimd.tensor_add(pm2[0][:], pm2[0][:], pm2[1][:])
            nc.vector.tensor_add(pm2[2][:], pm2[2][:], pm2[3][:])
            nc.gpsimd.tensor_add(pm2[0][:], pm2[0][:], pm2[2][:])

            # ---- cst4 row via thin ones-matmul; usim[65, 1024] ----
            cst_ps = psum.tile([1, 1024], F32, tag="cst_ps")
            cst = pool.tile([1, C], F32, tag="cst")
            usim = psum.tile([BL + 1, 1024], F32, tag="usim")
            for h in range(2):
                pcols = slice(h * SP, h * SP + CW[h])
                nc.tensor.matmul(cst_ps[:, pcols], lhsT=ones_col[:],
                                 rhs=pm2[0][:, pcols], start=True, stop=True)
                nc.vector.tensor_copy(cst[:, pcols], cst_ps[:, pcols])
                for t in range(DT):
                    nc.tensor.matmul(usim[:, pcols], lhsT=x2a[t][:],
                                     rhs=E[t][:, pcols],
                                     start=(t == 0), stop=False)
                for t in range(DT):
                    nc.tensor.matmul(usim[:, pcols], lhsT=xa[t][:],
                                     rhs=W2[t][:, pcols],
                                     start=False, stop=False)
                nc.tensor.matmul(usim[:, pcols], lhsT=q_row[:],
                                 rhs=cst[:, pcols], start=False, stop=True)

            # ---- -1/S via [125, 8]-shaped reciprocal (parallel lanes);
            # runs concurrently with the main usim matmuls ----
            s_dr = dram.tile([1, C], F32, tag="s_dr")
            s_row = pool.tile([1, C], F32, tag="s_row")
            nc.scalar.copy(s_row[:], S_ps[:, :C])
            nc.sync.dma_start(s_dr[:], s_row[:])
            s_sq = pool.tile([125, 8], F32, tag="s_sq")
            nc.sync.dma_start(s_sq[:], s_dr[:].rearrange("1 (p f) -> p f", f=8))
            nc.vector.reciprocal(s_sq[:], s_sq[:])
            nc.vector.tensor_scalar_mul(s_sq[:], s_sq[:], -1.0)
            nnr_dr = dram.tile([1, C], F32, tag="nnr_dr")
            nc.sync.dma_start(nnr_dr[:].rearrange("1 (p f) -> p f", f=8),
                              s_sq[:])
            nrB = pool.tile([BL, C], F32, tag="nrB_sb")
            nc.sync.dma_start(nrB[:], nnr_dr[:].to_broadcast([BL, C]))
            nsimi = pool.tile([BL, C], F32, tag="nsimi")
            nc.vector.tensor_mul(nsimi[:], usim[:BL, :C], nrB[:])

            # ---- top-10 smallest distances (largest neg_simi) ----
            v1 = pool.tile([BL, 8], F32, tag="v1")
            nc.vector.max(v1[:], nsimi[:])
            ns2 = pool.tile([BL, C], F32, tag="ns2")
            nc.vector.match_replace(ns2[:], v1[:], nsimi[:], -3.0e38)
            v2 = pool.tile([BL, 8], F32, tag="v2")
            nc.vector.max(v2[:], ns2[:])

            # conf = S10/v computed on negated values: (-S10)/(-v) = S10/v
            tk = pool.tile([BL, K], F32, tag="tk")
            nc.scalar.copy(tk[:, :8], v1[:])
            nc.scalar.copy(tk[:, 8:K], v2[:, :K - 8])
            s10 = pool.tile([BL, 1], F32, tag="s10")
            nc.vector.tensor_reduce(s10[:], tk[:], axis=X, op=Alu.add)
            rv = pool.tile([BL, K], F32, tag="rv")
            nc.vector.reciprocal(rv[:], tk[:])
            conf = pool.tile([BL, K], F32, tag="conf")
            nc.vector.tensor_scalar_mul(conf[:], rv[:], s10[:])
            nc.sync.dma_start(conf_d[:], conf[:])

            # ---- predict: label of the (first) argmin distance ----
            # eqm = -BIG*(nsimi==min) ; masked = eqm + label ; rowmin + BIG
            eqm = pool.tile([BL, C], F32, tag="eqm")
            nc.vector.tensor_scalar(eqm[:], nsimi[:], v1[:, 0:1], -BIG,
                                    op0=Alu.is_equal, op1=Alu.mult)
            masked = pool.tile([BL, C], F32, tag="masked")
            nc.gpsimd.tensor_add(masked[:], eqm[:], labelB[:])
            predf = pool.tile([BL, 1], F32, tag="predf")
            nc.vector.tensor_reduce(predf[:], masked[:], axis=X, op=Alu.min)
            predb = pool.tile([BL, 1], F32, tag="predb")
            nc.scalar.activation(predb[:], predf[:],
                                 mybir.ActivationFunctionType.Copy,
                                 bias=BIG, scale=1.0)
            predi = pool.tile([BL, 1], dt.int32, tag="predi")
            nc.vector.tensor_copy(predi[:], predb[:])
            nc.sync.dma_start(pred_d[:], predi[:])

    if split_waits:
        _split_sync_waits(nc)
    return nc


def make_in_maps(x, protos, ex2, ex1, cls_num, proto_label):
    x = np.asarray(x, dtype=np.float32)
    protos = np.asarray(protos, dtype=np.float32)
    ex2 = np.asarray(ex2, dtype=np.float32)
    ex1 = np.asarray(ex1, dtype=np.float32)
    sqn = np.sqrt(np.asarray(cls_num).astype(np.float32))[:, None]
    label_f = np.ascontiguousarray(np.asarray(proto_label).astype(np.float32)[None, :])
    pm2T = np.ascontiguousarray((-2.0 * protos).T)
    ne2T = np.ascontiguousarray((sqn * ex2).T)
    e1T = np.ascontiguousarray(ex1.T)
    xT = np.ascontiguousarray(x.T)
    in_maps = []
    for g in range(NCORES):
        in_maps.append({
            "pm2T": pm2T,
            "ne2T": ne2T,
            "e1T": e1T,
            "xT": np.ascontiguousarray(xT[:, g * BL:(g + 1) * BL]),
            "labelr": label_f,
        })
    return in_maps


def assemble(results):
    conf = np.concatenate([results[g]["conf"] for g in range(NCORES)], axis=0)
    pred = np.concatenate([results[g]["pred"][:, 0] for g in range(NCORES)],
                          axis=0).astype(np.int32)
    return pred, conf


def kernel(x, protos, ex2, ex1, cls_num, proto_label, k_nearest=K,
           _trace=False, _tmpdir=None):
    from concourse.bass_utils import run_bass_kernel_spmd

    if "nc" not in _CACHE:
        _CACHE["nc"] = build_program()
    nc = _CACHE["nc"]
    in_maps = make_in_maps(x, protos, ex2, ex1, cls_num, proto_label)
    res = run_bass_kernel_spmd(nc, in_maps, core_ids=list(range(NCORES)),
                               trace=_trace, tmpdir=_tmpdir)
    _CACHE["last_res"] = res
    return assemble(res.results)
